# revision 34
# baseline (speedup 1.0000x reference)
# Trainium2 Bass kernel for nn_AxialAttention (8 NeuronCores, W-parallel).
#
# Sharding: the W axis (axis=2, the vmapped axis) is split into 8 contiguous
# slices of 32 columns, one per core. Every part of the computation (the four
# 1x1-conv GEMMs, the per-(head, w) axial attention, the embedding terms) is
# independent across w, so there are no collectives; the small weight matrices
# and embedding tables are replicated to every core.
#
# Per-core math for one w column (all heads):
#   qsT[x, (h c)] = query[:, :, w].T @ Wq.T     (fp8 DoubleRow, K=256/pass)
#   khT[x, (h c)] = key_[:, :, w].T @ Wk.T      (fp8 DoubleRow)
#   vh [(h c), x] = Wv @ value[:, :, w]         (bf16)
#   logits_h[C, c] = khT_h.T @ qsT_h + qe.T @ qsT + ke.T @ khT
#     (scales: qin/kin fp8 at 1x, Wq/Wk fp8 at 64x, qsT/khT stored fp8 at
#      16x -> logits accumulate 4096*(true logits); softmax's 1/sqrt(256)
#      is folded into that 4096)
#   E = exp(logits / 4096)        (max-subtraction unnecessary: |logits|<~2)
#   U_h = E_h.T @ [vh_h + ve | 1]          (ones column gives the softmax
#   attn_h = U_h[:, :256] / U_h[:, 256]     denominator for free)
#   out[:, :, w] = Wo @ attn                (bf16 GEMM, output stored bf16)
#
# fp8 (e4m3) is used only where quantization noise lands pre-softmax (the
# q/k path); the v/o path must stay bf16 (fp8 there measures 3.5e-2 vs the
# 2e-2 gate).
#
# Scheduling: the PE weight-load port is a co-bottleneck with the matmul
# stream, so the kernel is organized around hiding LDWEIGHTS:
#  - Heads are packed two-per-128-block (block order [0,1],[3,2],[4,5],[7,6]
#    via a host-side channel permutation of Wq/Wk/Wv/Wo), so each head-pair's
#    logits term is ONE full-array DoubleRow matmul (cross-head products land
#    in the partition half the other head doesn't use) and each head-pair's
#    attention-x-values product is ONE full-array matmul (the unused halves
#    of the exp tile are kept hard-zero by masked exp writes, so cross-head
#    rows contribute exactly 0). This halves both matmul and LDWEIGHTS count
#    versus 64-wide per-head tiles.
#  - fp8 DoubleRow matmuls (256-column LDWEIGHTS that can't hide behind
#    another DR matmul) are emitted strictly interleaved with bf16 matmuls
#    (o/v projections) whose streams cover the load: phase A of iteration i
#    alternates o(i-2) with the q/k projections of i plus the attention
#    matmuls of i-1; phase B alternates the v projection of i with the
#    logits matmuls of i.
#  - Output is written bf16 on the otherwise-idle GpSimd DGE ring; PSUM->SBUF
#    evacuations are split across the Scalar/Vector engines.

import numpy as np

H = 8          # heads
QK = 64        # per-head qk/vo channels
C = 512        # io channels
X = 256        # spatial H (attention contraction axis)
W = 256        # spatial W (vmapped axis, sharded)
N_CORES = 8
WC = W // N_CORES   # w columns per core
PAIRS = WC // 2
NB = 4         # head-pair blocks (2 heads per 128 channels)

_CACHE = {}


def _build_program():
    import concourse.mybir as mybir
    import concourse.tile as tile
    from concourse import bacc

    f32 = mybir.dt.float32
    bf16 = mybir.dt.bfloat16
    fp8 = mybir.dt.float8e4
    AF = mybir.ActivationFunctionType
    DR = mybir.MatmulPerfMode.DoubleRow

    nc = bacc.Bacc("TRN2", target_bir_lowering=False, debug=False,
                   num_devices=N_CORES)

    qin = nc.dram_tensor("qin", [PAIRS, C, 2, X], fp8, kind="ExternalInput").ap()
    kin = nc.dram_tensor("kin", [PAIRS, C, 2, X], fp8, kind="ExternalInput").ap()
    vin = nc.dram_tensor("vin", [PAIRS, C, 2, X], bf16, kind="ExternalInput").ap()
    wqt = nc.dram_tensor("wqt", [C, C], fp8, kind="ExternalInput").ap()
    wkt = nc.dram_tensor("wkt", [C, C], fp8, kind="ExternalInput").ap()
    wvt = nc.dram_tensor("wvt", [C, C], bf16, kind="ExternalInput").ap()
    wot = nc.dram_tensor("wot", [C, C], bf16, kind="ExternalInput").ap()
    qe2 = nc.dram_tensor("qe2", [X, 2 * QK], fp8, kind="ExternalInput").ap()
    ke2 = nc.dram_tensor("ke2", [X, 2 * QK], fp8, kind="ExternalInput").ap()
    vet = nc.dram_tensor("vet", [128, 2, X], f32, kind="ExternalInput").ap()
    # pair-major so each 128-channel store is one contiguous 128KB block
    # (the [C, WC, X] layout forces 512B scattered writes that cap the
    # output DGE queue at ~44 GB/s); untangled on the host for free.
    out = nc.dram_tensor("out", [PAIRS, C, 2, X], bf16,
                         kind="ExternalOutput").ap()

    KT = C // 128   # 4 contraction tiles of the channel dim
    XT = X // 128   # 2 tiles of the spatial-x dim

    with tile.TileContext(nc) as tc:
        with (
            tc.tile_pool(name="consts", bufs=1) as consts,
            tc.tile_pool(name="inp", bufs=4) as inp,
            tc.tile_pool(name="qkt", bufs=2) as qkt,
            tc.tile_pool(name="mid", bufs=2) as mid,
            tc.tile_pool(name="small", bufs=8) as small,
            tc.tile_pool(name="psQK", bufs=2, space="PSUM") as psQK,
            tc.tile_pool(name="psOV", bufs=2, space="PSUM") as psOV,
            tc.tile_pool(name="psL", bufs=2, space="PSUM") as psL,
            tc.tile_pool(name="psU", bufs=2, space="PSUM") as psU,
        ):
            def load_inputs(pair):
                q_t = inp.tile([128, KT, 2, X], fp8, tag="q_t")
                nc.sync.dma_start(
                    q_t[:], qin[pair].rearrange("(kt p) w x -> p kt (w x)", p=128))
                k_t = inp.tile([128, KT, 2, X], fp8, tag="k_t")
                nc.sync.dma_start(
                    k_t[:], kin[pair].rearrange("(kt p) w x -> p kt (w x)", p=128))
                v_t = inp.tile([128, KT, 2, X], bf16, tag="v_t")
                nc.sync.dma_start(
                    v_t[:], vin[pair].rearrange("(kt p) w x -> p kt (w x)", p=128))
                return q_t, k_t, v_t

            # pair-0 inputs first so the PE can start ASAP; each dma_start
            # costs ~600ns of issue time on its DGE queue, so loads are
            # halved (not quartered) and ordered by first use: q/k k-tiles
            # 0-1 feed the first DR matmuls, v trails.
            q0 = inp.tile([128, KT, 2, X], fp8, tag="q_t")
            k0 = inp.tile([128, KT, 2, X], fp8, tag="k_t")
            v0 = inp.tile([128, KT, 2, X], bf16, tag="v_t")
            qr0 = qin[0].rearrange("(h p) w x -> p h (w x)", p=128)
            kr0 = kin[0].rearrange("(h p) w x -> p h (w x)", p=128)
            vr0 = vin[0].rearrange("(h p) w x -> p h (w x)", p=128)
            nc.sync.dma_start(q0[:, 0:2, :, :], qr0[:, 0:2, :])
            nc.sync.dma_start(k0[:, 0:2, :, :], kr0[:, 0:2, :])
            nc.sync.dma_start(q0[:, 2:4, :, :], qr0[:, 2:4, :])
            nc.sync.dma_start(k0[:, 2:4, :, :], kr0[:, 2:4, :])
            nc.sync.dma_start(v0[:, 0:2, :, :], vr0[:, 0:2, :])
            nc.sync.dma_start(v0[:, 2:4, :, :], vr0[:, 2:4, :])
            prefetched = (q0, k0, v0)

            # wq first (the first matmul needs only its kt 0-1 half), then wk.
            wq_sb = consts.tile([128, KT, C], fp8)
            wqr = wqt.rearrange("(kt p) o -> p kt o", p=128)
            nc.scalar.dma_start(wq_sb[:, 0:2, :], wqr[:, 0:2, :])
            nc.scalar.dma_start(wq_sb[:, 2:4, :], wqr[:, 2:4, :])
            wk_sb = consts.tile([128, KT, C], fp8)
            nc.scalar.dma_start(wk_sb[:], wkt.rearrange("(kt p) o -> p kt o", p=128))
            # later-used constants go on the idle GpSimd ring so the scalar
            # ring only carries what the first matmuls need; ordered by first
            # use (emb tables feed the warmup logits, wv is split so the
            # first v matmul only waits half of it, wo isn't needed until
            # iteration 2).
            ke_sb = consts.tile([128, XT, 2 * QK], fp8)
            nc.gpsimd.dma_start(ke_sb[:], ke2.rearrange("(xt p) m -> p xt m", p=128))
            qe_sb = consts.tile([128, XT, 2 * QK], fp8)
            nc.gpsimd.dma_start(qe_sb[:], qe2.rearrange("(xt p) m -> p xt m", p=128))
            wv_sb = consts.tile([128, KT, C], bf16)
            wvr = wvt.rearrange("(kt p) o -> p kt o", p=128)
            nc.gpsimd.dma_start(wv_sb[:, 0:2, :], wvr[:, 0:2, :])
            nc.gpsimd.dma_start(wv_sb[:, 2:4, :], wvr[:, 2:4, :])
            ve_sb = consts.tile([128, 2, X], f32)   # dup'd over head-half and w
            nc.gpsimd.dma_start(ve_sb[:], vet[:])
            wo_sb = consts.tile([128, KT, C], bf16)
            nc.gpsimd.dma_start(wo_sb[:], wot.rearrange("(kt p) o -> p kt o", p=128))

            # vplus double-buffer with the ones columns filled exactly once
            # (they never change; pool rotation would clobber them).
            vplus_bufs = []
            for b in range(2):
                vb = mid.tile([128, NB, 2, X + 2], bf16, tag=f"vplus{b}")
                nc.vector.memset(vb[:, :, :, X:X + 2], 1.0)
                vplus_bufs.append(vb)

            # exp tiles: the off-half of each head-pair block must stay ZERO
            # so the merged (full-K) attention matmul gets exactly-zero
            # cross-head contributions. Memset once; exp only writes the
            # valid half of each block.
            e_bufs = []
            for b in range(2):
                eb = mid.tile([128, 2, NB, 128], bf16, tag=f"e{b}")
                nc.vector.memset(eb[0:QK, :, :, QK:128], 0.0)
                nc.vector.memset(eb[QK:128, :, :, 0:QK], 0.0)
                e_bufs.append(eb)

            # ---------------- per-iteration emission helpers ----------------
            # Each helper returns a list of thunks; calling a thunk emits ONE
            # PE matmul (plus any trailing non-PE ops tied to it). Emission
            # order = scheduler priority = (modulo readiness) PE issue order.

            def qk_thunks(q_t, k_t, qsT, khT, kp_first=False, pool_plan=None):
                # kp_first: emit [q_kp0, k_kp0, q_kp1, k_kp1] per (wi, xt) so
                # the k-tile-23 matmuls trail the k-tile-01 ones, matching
                # the DMA arrival order of the input chunks (warmup only).
                # pool_plan: per-group (pool, tag) override so iteration 0
                # can spread its 8 groups across all four PSUM pools.
                th = []
                gi = 0
                for wi in range(2):
                    for xt in range(XT):
                        for which, src, wsb, dstT in (("q", q_t, wq_sb, qsT),
                                                      ("k", k_t, wk_sb, khT)):
                            pool, tag = (pool_plan[gi] if pool_plan
                                         else (psQK, "qk"))
                            gi += 1
                            cell = {}
                            def t0(cell=cell, src=src, wsb=wsb, wi=wi, xt=xt,
                                   pool=pool, tag=tag):
                                p = pool.tile([128, C], f32, tag=tag)
                                cell["p"] = p
                                nc.tensor.matmul(
                                    p[:],
                                    src[:, 0:2, wi, xt * 128:(xt + 1) * 128],
                                    wsb[:, 0:2, :],
                                    start=True, stop=False, perf_mode=DR)
                            def t1(cell=cell, src=src, wsb=wsb, dstT=dstT,
                                   which=which, wi=wi, xt=xt):
                                p = cell["p"]
                                nc.tensor.matmul(
                                    p[:],
                                    src[:, 2:4, wi, xt * 128:(xt + 1) * 128],
                                    wsb[:, 2:4, :],
                                    start=False, stop=True, perf_mode=DR)
                                # high priority: the evac is the PSUM-bank
                                # release; with only 2 qk banks a queue delay
                                # here stalls the PE two groups later.
                                with tc.high_priority():
                                    if which == "q":
                                        nc.scalar.activation(
                                            dstT[:, wi, xt, :], p[:], AF.Copy,
                                            scale=0.25)
                                    else:
                                        nc.vector.tensor_scalar_mul(
                                            dstT[:, wi, xt, :], p[:], 0.25)
                            th += [t0, t1]
                if kp_first:
                    # [qa qb ka kb] -> [qa ka qb kb] within each (wi, xt)
                    th = [th[g + j] for g in range(0, len(th), 4)
                          for j in (0, 2, 1, 3)]
                return th

            def proj_thunks(src, wsb, on_group_done):
                # generic 4x4 bf16 projection: out-block ot accumulates kt 0..3
                th = []
                for ot in range(KT):
                    cell = {}
                    for kt in range(KT):
                        def f(cell=cell, ot=ot, kt=kt):
                            if kt == 0:
                                cell["p"] = psOV.tile([128, 2, X], f32, tag="ov", name="pov")
                            nc.tensor.matmul(
                                cell["p"][:],
                                wsb[:, kt, ot * 128:(ot + 1) * 128],
                                src[:, kt, :, :],
                                start=(kt == 0), stop=(kt == KT - 1))
                            if kt == KT - 1:
                                on_group_done(ot, cell["p"])
                        th.append(f)
                return th

            def v_thunks(v_t, vplus):
                def done(ot, psum):
                    nc.vector.tensor_add(
                        vplus[:, ot, :, 0:X], psum[:], ve_sb[:])
                return proj_thunks(v_t, wv_sb, done)

            def o_thunks(attn, pr):
                def done(ot, psum):
                    ob = small.tile([128, 2, X], bf16, tag="ob")
                    if ot % 2 == 0:
                        nc.scalar.activation(ob[:, 0, :], psum[:, 0, :], AF.Copy)
                        nc.vector.tensor_copy(ob[:, 1, :], psum[:, 1, :])
                    else:
                        nc.scalar.activation(ob[:, 1, :], psum[:, 1, :], AF.Copy)
                        nc.vector.tensor_copy(ob[:, 0, :], psum[:, 0, :])
                    nc.gpsimd.dma_start(
                        out[pr, ot * 128:(ot + 1) * 128, :, :], ob[:])
                return proj_thunks(attn, wo_sb, done)

            def logits_thunks(qsT, khT, e_t):
                # per wi: ke, qe emb DR matmuls + NB merged head-pair DR
                # matmuls accumulating into pl, then masked exp -> e_t.
                th = []
                for wi in range(2):
                    cell = {}
                    def t_ke(cell=cell, wi=wi):
                        pl = psL.tile([128, NB, 128], f32, tag="pl", name="pl")
                        cell["pl"] = pl
                        nc.tensor.matmul(
                            pl[:], ke_sb[:], khT[:, wi, :, :],
                            start=True, stop=False, perf_mode=DR)
                    def t_qe(cell=cell, wi=wi):
                        nc.tensor.matmul(
                            cell["pl"][:], qe_sb[:], qsT[:, wi, :, :],
                            start=False, stop=False, perf_mode=DR)
                    th += [t_ke, t_qe]
                    for p in range(NB):
                        def t_hp(cell=cell, wi=wi, p=p):
                            pl = cell["pl"]
                            nc.tensor.matmul(
                                pl[:, p, :],
                                khT[:, wi, :, p * 128:(p + 1) * 128],
                                qsT[:, wi, :, p * 128:(p + 1) * 128],
                                start=False, stop=(p == NB - 1),
                                perf_mode=DR)
                            if p == NB - 1:
                                nc.scalar.activation(
                                    e_t[0:QK, wi, :, 0:QK],
                                    pl[0:QK, :, 0:QK], AF.Exp,
                                    scale=1.0 / 4096.0)
                                nc.scalar.activation(
                                    e_t[QK:128, wi, :, QK:128],
                                    pl[QK:128, :, QK:128], AF.Exp,
                                    scale=1.0 / 4096.0)
                        th.append(t_hp)
                return th

            def pu_thunks(e_t, vplus, attn):
                # p-major: the o projection consumes attn k-tile 0 (= block
                # p=0, both wi) first, so evacuate in that order.
                th = []
                for p in range(NB):
                    for wi in range(2):
                        def t(wi=wi, p=p):
                            pu = psU.tile([128, X + 2], f32, tag="pu")
                            nc.tensor.matmul(
                                pu[:],
                                e_t[:, wi, p, :],
                                vplus[:, p, wi, :],
                                start=True, stop=True)
                            # high priority: recip -> scale is the psU bank
                            # release chain; a queue delay stalls the pu
                            # matmul two tiles later.
                            with tc.high_priority():
                                recip = small.tile([128, 1], f32, tag="recip")
                                nc.vector.reciprocal(recip[:], pu[:, X:X + 1])
                                if (2 * wi + p) % 4 != 3:  # scalar is lighter
                                    nc.scalar.activation(
                                        attn[:, p, wi, :],
                                        pu[:, 0:X], AF.Copy, scale=recip[:])
                                else:
                                    nc.vector.tensor_scalar_mul(
                                        attn[:, p, wi, :], pu[:, 0:X], recip[:])
                        th.append(t)
                return th

            def interleave(big_a, big_b, extra=(), every=4):
                # alternate big_a/big_b; insert one `extra` thunk after every
                # `every` big thunks.
                n = max(len(big_a), len(big_b))
                ei = 0
                cnt = 0
                for i in range(n):
                    for lst in (big_a, big_b):
                        if i < len(lst):
                            lst[i]()
                            cnt += 1
                            if cnt % every == 0 and ei < len(extra):
                                extra[ei]()
                                ei += 1
                while ei < len(extra):
                    extra[ei]()
                    ei += 1

            # ---------------- the software pipeline ----------------
            # iteration i emits: phase A = o(i-2) x qk(i) with pu(i-1)
            # sprinkled; phase B = v(i) x logits(i).
            e_hist = {}
            vplus_hist = {}
            attn_hist = {}

            for it in range(PAIRS + 2):
                if it < PAIRS:
                    q_t, k_t, v_t = prefetched if it == 0 else load_inputs(it)
                    qsT = qkt.tile([128, 2, XT, C], fp8, tag="qsT")
                    khT = qkt.tile([128, 2, XT, C], fp8, tag="khT")
                    vplus = vplus_bufs[it % 2]
                    vplus_hist[it] = vplus
                    e_t = e_bufs[it % 2]
                    e_hist[it] = e_t
                    plan0 = [(psQK, "qk"), (psQK, "qk"), (psOV, "ov"),
                             (psOV, "ov"), (psL, "pl"), (psL, "pl"),
                             (psU, "pu"), (psU, "pu")]
                    qk_th = qk_thunks(q_t, k_t, qsT, khT,
                                      kp_first=(it == 1),
                                      pool_plan=(plan0 if it == 0 else None))
                    v_th = v_thunks(v_t, vplus)
                    lg_th = logits_thunks(qsT, khT, e_t)
                else:
                    qk_th, v_th, lg_th = [], [], []

                if it >= 2:
                    attn = attn_hist.pop(it - 2)
                    o_th = o_thunks(attn, it - 2)
                else:
                    o_th = []

                if 1 <= it <= PAIRS:
                    e_p = e_hist.pop(it - 1)
                    vplus_p = vplus_hist.pop(it - 1)
                    attn_n = mid.tile([128, NB, 2, X], bf16, tag="attn")
                    attn_hist[it - 1] = attn_n
                    pu_th = pu_thunks(e_p, vplus_p, attn_n)
                else:
                    pu_th = []

                if it == 0:
                    # warmup: the PE queue is FIFO, so nothing that waits on
                    # late DMA may be emitted early. All eight k-tile-01
                    # matmuls need only the first two input chunks -> run
                    # them first (their PSUM tiles are spread over all four
                    # pools, see pool_plan), then the k-tile-23 matmuls,
                    # then pair the v matmuls against the logits work.
                    for t in qk_th[0::2]:
                        t()
                    for t in qk_th[1::2]:
                        t()
                    interleave(v_th, lg_th)
                elif it == 1:
                    # pair-1 inputs are still streaming in; bridge the wait
                    # with pu(0), whose inputs are already on-chip.
                    for t in pu_th:
                        t()
                    dr_all = qk_th[:8] + lg_th[:6] + qk_th[8:] + lg_th[6:]
                    interleave(v_th, dr_all)
                else:
                    interleave(o_th, qk_th, pu_th, every=4)
                    interleave(v_th, lg_th)

    nc.compile()
    return nc


def _get_program():
    if "nc" not in _CACHE:
        _CACHE["nc"] = _build_program()
    return _CACHE["nc"]


def _make_in_maps(query, key_, value, Wq, Wk, Wv, Wo, q_emb, k_emb, v_emb):
    import ml_dtypes
    bf16 = ml_dtypes.bfloat16
    fp8 = ml_dtypes.float8_e4m3

    def q8(a, scale):
        return np.ascontiguousarray(
            np.clip(a * np.float32(scale), -240, 240).astype(fp8))

    # Head-pair channel permutation: 128-blocks hold heads [0,1],[3,2],
    # [4,5],[7,6]; the first head of a block owns partitions/block-columns
    # 0:64, the second 64:128. Applied to Wq/Wk/Wv output channels and Wo
    # input channels so the merged head-pair matmuls read/write contiguous
    # 128-blocks.
    vperm = np.arange(C).reshape(C // 128, 2, QK)[:, [0, 1], :].copy()
    vperm[1::2] = vperm[1::2][:, [1, 0], :]
    rowperm = vperm.reshape(-1)

    # Scale plan (logits accumulate 4096x, undone in the exp activation):
    #   qin/kin fp8 at 1x; Wq/Wk fp8 at 64x -> psum 64x; evac scale 0.25
    #   -> qsT/khT fp8 at 16x; head-pair term 256x = 4096 * (1/16 softmax).
    #   q_emb fp8 at 16x (pairs with qsT); k_emb fp8 at 256x (pairs with khT,
    #   no softmax scale on the k.ke term).
    wqt = q8(Wq[rowperm].T, 64.0)
    wkt = q8(Wk[rowperm].T, 64.0)
    wvt = np.ascontiguousarray(Wv[rowperm].T.astype(bf16))
    wot = np.ascontiguousarray(Wo.T[rowperm].astype(bf16))
    qe2 = q8(np.concatenate([q_emb, q_emb], axis=1), 16.0)
    ke2 = q8(np.concatenate([k_emb, k_emb], axis=1), 256.0)
    # ve dup'd over the two head-halves and the two w columns: [128, 2, X]
    ve1 = np.concatenate([v_emb.T, v_emb.T], axis=0)          # (128, X)
    vet = np.ascontiguousarray(
        np.broadcast_to(ve1[:, None, :], (128, 2, X)).astype(np.float32))

    def shard8(a, ws):
        # (C, X, WC) -> [pair, c, w, x] contiguous fp8
        return q8(
            a[:, :, ws].reshape(C, X, PAIRS, 2).transpose(2, 0, 3, 1), 1.0)

    def shardb(a, ws):
        return np.ascontiguousarray(
            a[:, :, ws].reshape(C, X, PAIRS, 2).transpose(2, 0, 3, 1).astype(bf16))

    in_maps = []
    for c in range(N_CORES):
        ws = slice(c * WC, (c + 1) * WC)
        in_maps.append({
            "qin": shard8(query, ws),
            "kin": shard8(key_, ws),
            "vin": shardb(value, ws),
            "wqt": wqt, "wkt": wkt, "wvt": wvt, "wot": wot,
            "qe2": qe2, "ke2": ke2, "vet": vet,
        })
    return in_maps


def _run(in_maps, trace=False):
    from concourse.bass_utils import run_bass_kernel_spmd
    nc = _get_program()
    return run_bass_kernel_spmd(nc, in_maps, list(range(N_CORES)), trace=trace)


def kernel(query, key_, value, Wq, Wk, Wv, Wo, q_emb, k_emb, v_emb):
    args = (query, key_, value, Wq, Wk, Wv, Wo, q_emb, k_emb, v_emb)
    in_maps = _make_in_maps(*[np.ascontiguousarray(a, np.float32) for a in args])
    res = _run(in_maps, trace=False)
    out = np.empty((C, X, W), np.float32)
    for c in range(N_CORES):
        # device layout [PAIRS, C, 2, X] -> (C, X, WC) with w = pair*2+wi
        arr = res.results[c]["out"].astype(np.float32)
        out[:, :, c * WC:(c + 1) * WC] = \
            arr.transpose(1, 3, 0, 2).reshape(C, X, WC)
    return out


# revision 39
# speedup vs baseline: 1.1827x; 1.1827x over previous
# Trainium2 Bass kernel for nn_AxialAttention (8 NeuronCores, W-parallel).
#
# Sharding: the W axis (axis=2, the vmapped axis) is split into 8 contiguous
# slices of 32 columns, one per core. Every part of the computation (the four
# 1x1-conv GEMMs, the per-(head, w) axial attention, the embedding terms) is
# independent across w, so there are no collectives; the small weight matrices
# and embedding tables are replicated to every core.
#
# Per-core math for one w column (all heads):
#   qsT[x, (h c)] = query[:, :, w].T @ Wq.T     (fp8 DoubleRow, K=256/pass)
#   khT[x, (h c)] = key_[:, :, w].T @ Wk.T      (fp8 DoubleRow)
#   vh [(h c), x] = Wv @ value[:, :, w]         (bf16)
#   logits_h[C, c] = khT_h.T @ qsT_h + qe.T @ qsT + ke.T @ khT
#     (scales: qin/kin fp8 at 1x, Wq/Wk fp8 at 64x, qsT/khT stored fp8 at
#      16x -> logits accumulate 4096*(true logits); softmax's 1/sqrt(256)
#      is folded into that 4096)
#   E = exp(logits / 4096)        (max-subtraction unnecessary: |logits|<~2)
#   U_h = E_h.T @ [vh_h + ve | 1]          (ones column gives the softmax
#   attn_h = U_h[:, :256] / U_h[:, 256]     denominator for free)
#   out[:, :, w] = Wo @ attn                (bf16 GEMM, output stored bf16)
#
# fp8 (e4m3) is used only where quantization noise lands pre-softmax (the
# q/k path); the v/o path must stay bf16 (fp8 there measures 3.5e-2 vs the
# 2e-2 gate).
#
# Scheduling: the PE weight-load port is a co-bottleneck with the matmul
# stream, so the kernel is organized around hiding LDWEIGHTS:
#  - Heads are packed two-per-128-block (block order [0,1],[3,2],[4,5],[7,6]
#    via a host-side channel permutation of Wq/Wk/Wv/Wo), so each head-pair's
#    logits term is ONE full-array DoubleRow matmul (cross-head products land
#    in the partition half the other head doesn't use) and each head-pair's
#    attention-x-values product is ONE full-array matmul (the unused halves
#    of the exp tile are kept hard-zero by masked exp writes, so cross-head
#    rows contribute exactly 0). This halves both matmul and LDWEIGHTS count
#    versus 64-wide per-head tiles.
#  - fp8 DoubleRow matmuls (256-column LDWEIGHTS that can't hide behind
#    another DR matmul) are emitted strictly interleaved with bf16 matmuls
#    (o/v projections) whose streams cover the load: phase A of iteration i
#    alternates o(i-2) with the q/k projections of i plus the attention
#    matmuls of i-1; phase B alternates the v projection of i with the
#    logits matmuls of i.
#  - Output is written bf16 on the otherwise-idle GpSimd DGE ring; PSUM->SBUF
#    evacuations are split across the Scalar/Vector engines.

import numpy as np

H = 8          # heads
QK = 64        # per-head qk/vo channels
C = 512        # io channels
X = 256        # spatial H (attention contraction axis)
W = 256        # spatial W (vmapped axis, sharded)
N_CORES = 8
WC = W // N_CORES   # w columns per core
PAIRS = WC // 2
NB = 4         # head-pair blocks (2 heads per 128 channels)

_CACHE = {}


def _build_program():
    import concourse.mybir as mybir
    import concourse.tile as tile
    from concourse import bacc

    f32 = mybir.dt.float32
    bf16 = mybir.dt.bfloat16
    fp8 = mybir.dt.float8e4
    AF = mybir.ActivationFunctionType
    DR = mybir.MatmulPerfMode.DoubleRow

    nc = bacc.Bacc("TRN2", target_bir_lowering=False, debug=False,
                   num_devices=N_CORES)

    qin = nc.dram_tensor("qin", [PAIRS, C, 2, X], fp8, kind="ExternalInput").ap()
    kin = nc.dram_tensor("kin", [PAIRS, C, 2, X], fp8, kind="ExternalInput").ap()
    vin = nc.dram_tensor("vin", [PAIRS, C, 2, X], bf16, kind="ExternalInput").ap()
    wqt = nc.dram_tensor("wqt", [C, C], fp8, kind="ExternalInput").ap()
    wkt = nc.dram_tensor("wkt", [C, C], fp8, kind="ExternalInput").ap()
    wvt = nc.dram_tensor("wvt", [C, C], bf16, kind="ExternalInput").ap()
    wot = nc.dram_tensor("wot", [C, C], bf16, kind="ExternalInput").ap()
    qe2 = nc.dram_tensor("qe2", [X, 2 * QK], fp8, kind="ExternalInput").ap()
    ke2 = nc.dram_tensor("ke2", [X, 2 * QK], fp8, kind="ExternalInput").ap()
    vet = nc.dram_tensor("vet", [128, 2, X], f32, kind="ExternalInput").ap()
    out = nc.dram_tensor("out", [C, WC, X], bf16, kind="ExternalOutput").ap()

    KT = C // 128   # 4 contraction tiles of the channel dim
    XT = X // 128   # 2 tiles of the spatial-x dim

    with tile.TileContext(nc) as tc:
        with (
            tc.tile_pool(name="consts", bufs=1) as consts,
            tc.tile_pool(name="inp", bufs=4) as inp,
            tc.tile_pool(name="qkt", bufs=2) as qkt,
            tc.tile_pool(name="mid", bufs=2) as mid,
            tc.tile_pool(name="small", bufs=8) as small,
            tc.tile_pool(name="psQK", bufs=2, space="PSUM") as psQK,
            tc.tile_pool(name="psOV", bufs=2, space="PSUM") as psOV,
            tc.tile_pool(name="psL", bufs=2, space="PSUM") as psL,
            tc.tile_pool(name="psU", bufs=2, space="PSUM") as psU,
        ):
            def load_inputs(pair):
                q_t = inp.tile([128, KT, 2, X], fp8, tag="q_t")
                nc.sync.dma_start(
                    q_t[:], qin[pair].rearrange("(kt p) w x -> p kt (w x)", p=128))
                k_t = inp.tile([128, KT, 2, X], fp8, tag="k_t")
                nc.sync.dma_start(
                    k_t[:], kin[pair].rearrange("(kt p) w x -> p kt (w x)", p=128))
                v_t = inp.tile([128, KT, 2, X], bf16, tag="v_t")
                nc.sync.dma_start(
                    v_t[:], vin[pair].rearrange("(kt p) w x -> p kt (w x)", p=128))
                return q_t, k_t, v_t

            # pair-0 inputs first so the PE can start ASAP; each dma_start
            # costs ~600ns of issue time on its DGE queue, so loads are
            # halved (not quartered) and ordered by first use: q/k k-tiles
            # 0-1 feed the first DR matmuls, v trails.
            q0 = inp.tile([128, KT, 2, X], fp8, tag="q_t")
            k0 = inp.tile([128, KT, 2, X], fp8, tag="k_t")
            v0 = inp.tile([128, KT, 2, X], bf16, tag="v_t")
            qr0 = qin[0].rearrange("(h p) w x -> p h (w x)", p=128)
            kr0 = kin[0].rearrange("(h p) w x -> p h (w x)", p=128)
            vr0 = vin[0].rearrange("(h p) w x -> p h (w x)", p=128)
            nc.sync.dma_start(q0[:, 0:2, :, :], qr0[:, 0:2, :])
            nc.sync.dma_start(k0[:, 0:2, :, :], kr0[:, 0:2, :])
            nc.sync.dma_start(q0[:, 2:4, :, :], qr0[:, 2:4, :])
            nc.sync.dma_start(k0[:, 2:4, :, :], kr0[:, 2:4, :])
            nc.sync.dma_start(v0[:, 0:2, :, :], vr0[:, 0:2, :])
            nc.sync.dma_start(v0[:, 2:4, :, :], vr0[:, 2:4, :])
            prefetched = (q0, k0, v0)

            # wq first (the first matmul needs only its kt 0-1 half), then wk.
            wq_sb = consts.tile([128, KT, C], fp8)
            wqr = wqt.rearrange("(kt p) o -> p kt o", p=128)
            nc.scalar.dma_start(wq_sb[:, 0:2, :], wqr[:, 0:2, :])
            nc.scalar.dma_start(wq_sb[:, 2:4, :], wqr[:, 2:4, :])
            wk_sb = consts.tile([128, KT, C], fp8)
            nc.scalar.dma_start(wk_sb[:], wkt.rearrange("(kt p) o -> p kt o", p=128))
            # later-used constants go on the idle GpSimd ring so the scalar
            # ring only carries what the first matmuls need; ordered by first
            # use (emb tables feed the warmup logits, wv is split so the
            # first v matmul only waits half of it, wo isn't needed until
            # iteration 2).
            ke_sb = consts.tile([128, XT, 2 * QK], fp8)
            nc.gpsimd.dma_start(ke_sb[:], ke2.rearrange("(xt p) m -> p xt m", p=128))
            qe_sb = consts.tile([128, XT, 2 * QK], fp8)
            nc.gpsimd.dma_start(qe_sb[:], qe2.rearrange("(xt p) m -> p xt m", p=128))
            wv_sb = consts.tile([128, KT, C], bf16)
            wvr = wvt.rearrange("(kt p) o -> p kt o", p=128)
            nc.gpsimd.dma_start(wv_sb[:, 0:2, :], wvr[:, 0:2, :])
            nc.gpsimd.dma_start(wv_sb[:, 2:4, :], wvr[:, 2:4, :])
            ve_sb = consts.tile([128, 2, X], f32)   # dup'd over head-half and w
            nc.gpsimd.dma_start(ve_sb[:], vet[:])
            wo_sb = consts.tile([128, KT, C], bf16)
            nc.gpsimd.dma_start(wo_sb[:], wot.rearrange("(kt p) o -> p kt o", p=128))

            # vplus double-buffer with the ones columns filled exactly once
            # (they never change; pool rotation would clobber them).
            vplus_bufs = []
            for b in range(2):
                vb = mid.tile([128, NB, 2, X + 2], bf16, tag=f"vplus{b}")
                nc.vector.memset(vb[:, :, :, X:X + 2], 1.0)
                vplus_bufs.append(vb)

            # exp tiles: the off-half of each head-pair block must stay ZERO
            # so the merged (full-K) attention matmul gets exactly-zero
            # cross-head contributions. Memset once; exp only writes the
            # valid half of each block.
            e_bufs = []
            for b in range(2):
                eb = mid.tile([128, 2, NB, 128], bf16, tag=f"e{b}")
                nc.vector.memset(eb[0:QK, :, :, QK:128], 0.0)
                nc.vector.memset(eb[QK:128, :, :, 0:QK], 0.0)
                e_bufs.append(eb)

            # ---------------- per-iteration emission helpers ----------------
            # Each helper returns a list of thunks; calling a thunk emits ONE
            # PE matmul (plus any trailing non-PE ops tied to it). Emission
            # order = scheduler priority = (modulo readiness) PE issue order.

            def qk_thunks(q_t, k_t, qsT, khT, kp_first=False, pool_plan=None):
                # kp_first: emit [q_kp0, k_kp0, q_kp1, k_kp1] per (wi, xt) so
                # the k-tile-23 matmuls trail the k-tile-01 ones, matching
                # the DMA arrival order of the input chunks (warmup only).
                # pool_plan: per-group (pool, tag) override so iteration 0
                # can spread its 8 groups across all four PSUM pools.
                th = []
                gi = 0
                for wi in range(2):
                    for xt in range(XT):
                        for which, src, wsb, dstT in (("q", q_t, wq_sb, qsT),
                                                      ("k", k_t, wk_sb, khT)):
                            pool, tag = (pool_plan[gi] if pool_plan
                                         else (psQK, "qk"))
                            gi += 1
                            cell = {}
                            def t0(cell=cell, src=src, wsb=wsb, wi=wi, xt=xt,
                                   pool=pool, tag=tag):
                                p = pool.tile([128, C], f32, tag=tag)
                                cell["p"] = p
                                nc.tensor.matmul(
                                    p[:],
                                    src[:, 0:2, wi, xt * 128:(xt + 1) * 128],
                                    wsb[:, 0:2, :],
                                    start=True, stop=False, perf_mode=DR)
                            def t1(cell=cell, src=src, wsb=wsb, dstT=dstT,
                                   which=which, wi=wi, xt=xt):
                                p = cell["p"]
                                nc.tensor.matmul(
                                    p[:],
                                    src[:, 2:4, wi, xt * 128:(xt + 1) * 128],
                                    wsb[:, 2:4, :],
                                    start=False, stop=True, perf_mode=DR)
                                # high priority: the evac is the PSUM-bank
                                # release; with only 2 qk banks a queue delay
                                # here stalls the PE two groups later.
                                with tc.high_priority():
                                    if which == "q":
                                        nc.scalar.activation(
                                            dstT[:, wi, xt, :], p[:], AF.Copy,
                                            scale=0.25)
                                    else:
                                        nc.vector.tensor_scalar_mul(
                                            dstT[:, wi, xt, :], p[:], 0.25)
                            th += [t0, t1]
                if kp_first:
                    # [qa qb ka kb] -> [qa ka qb kb] within each (wi, xt)
                    th = [th[g + j] for g in range(0, len(th), 4)
                          for j in (0, 2, 1, 3)]
                return th

            def proj_thunks(src, wsb, on_group_done):
                # generic 4x4 bf16 projection: out-block ot accumulates kt 0..3
                th = []
                for ot in range(KT):
                    cell = {}
                    for kt in range(KT):
                        def f(cell=cell, ot=ot, kt=kt):
                            if kt == 0:
                                cell["p"] = psOV.tile([128, 2, X], f32, tag="ov", name="pov")
                            nc.tensor.matmul(
                                cell["p"][:],
                                wsb[:, kt, ot * 128:(ot + 1) * 128],
                                src[:, kt, :, :],
                                start=(kt == 0), stop=(kt == KT - 1))
                            if kt == KT - 1:
                                on_group_done(ot, cell["p"])
                        th.append(f)
                return th

            def v_thunks(v_t, vplus):
                def done(ot, psum):
                    nc.vector.tensor_add(
                        vplus[:, ot, :, 0:X], psum[:], ve_sb[:])
                return proj_thunks(v_t, wv_sb, done)

            def o_thunks(attn, w0):
                def done(ot, psum):
                    ob = small.tile([128, 2, X], bf16, tag="ob")
                    if ot % 2 == 0:
                        nc.scalar.activation(ob[:, 0, :], psum[:, 0, :], AF.Copy)
                        nc.vector.tensor_copy(ob[:, 1, :], psum[:, 1, :])
                    else:
                        nc.scalar.activation(ob[:, 1, :], psum[:, 1, :], AF.Copy)
                        nc.vector.tensor_copy(ob[:, 0, :], psum[:, 0, :])
                    # one HWDGE queue caps at ~40-44 GB/s, which barely
                    # covers the 512KB/iteration of output: alternate
                    # between the gpsimd ring and the (post-startup idle)
                    # scalar ring so neither backs up.
                    ring = nc.gpsimd if ot % 2 == 0 else nc.scalar
                    ring.dma_start(
                        out[ot * 128:(ot + 1) * 128, w0:w0 + 2, :], ob[:])
                return proj_thunks(attn, wo_sb, done)

            def logits_thunks(qsT, khT, e_t):
                # per wi: ke, qe emb DR matmuls + NB merged head-pair DR
                # matmuls accumulating into pl, then masked exp -> e_t.
                th = []
                for wi in range(2):
                    cell = {}
                    def t_ke(cell=cell, wi=wi):
                        pl = psL.tile([128, NB, 128], f32, tag="pl", name="pl")
                        cell["pl"] = pl
                        nc.tensor.matmul(
                            pl[:], ke_sb[:], khT[:, wi, :, :],
                            start=True, stop=False, perf_mode=DR)
                    def t_qe(cell=cell, wi=wi):
                        nc.tensor.matmul(
                            cell["pl"][:], qe_sb[:], qsT[:, wi, :, :],
                            start=False, stop=False, perf_mode=DR)
                    th += [t_ke, t_qe]
                    for p in range(NB):
                        def t_hp(cell=cell, wi=wi, p=p):
                            pl = cell["pl"]
                            nc.tensor.matmul(
                                pl[:, p, :],
                                khT[:, wi, :, p * 128:(p + 1) * 128],
                                qsT[:, wi, :, p * 128:(p + 1) * 128],
                                start=False, stop=(p == NB - 1),
                                perf_mode=DR)
                            if p == NB - 1:
                                nc.scalar.activation(
                                    e_t[0:QK, wi, :, 0:QK],
                                    pl[0:QK, :, 0:QK], AF.Exp,
                                    scale=1.0 / 4096.0)
                                nc.scalar.activation(
                                    e_t[QK:128, wi, :, QK:128],
                                    pl[QK:128, :, QK:128], AF.Exp,
                                    scale=1.0 / 4096.0)
                        th.append(t_hp)
                return th

            def pu_thunks(e_t, vplus, attn):
                # p-major: the o projection consumes attn k-tile 0 (= block
                # p=0, both wi) first, so evacuate in that order.
                th = []
                for p in range(NB):
                    for wi in range(2):
                        def t(wi=wi, p=p):
                            pu = psU.tile([128, X + 2], f32, tag="pu")
                            nc.tensor.matmul(
                                pu[:],
                                e_t[:, wi, p, :],
                                vplus[:, p, wi, :],
                                start=True, stop=True)
                            # high priority: recip -> scale is the psU bank
                            # release chain; a queue delay stalls the pu
                            # matmul two tiles later.
                            with tc.high_priority():
                                recip = small.tile([128, 1], f32, tag="recip")
                                nc.vector.reciprocal(recip[:], pu[:, X:X + 1])
                                if (2 * wi + p) % 4 != 3:  # scalar is lighter
                                    nc.scalar.activation(
                                        attn[:, p, wi, :],
                                        pu[:, 0:X], AF.Copy, scale=recip[:])
                                else:
                                    nc.vector.tensor_scalar_mul(
                                        attn[:, p, wi, :], pu[:, 0:X], recip[:])
                        th.append(t)
                return th

            def interleave(big_a, big_b, extra=(), every=4):
                # alternate big_a/big_b; insert one `extra` thunk after every
                # `every` big thunks.
                n = max(len(big_a), len(big_b))
                ei = 0
                cnt = 0
                for i in range(n):
                    for lst in (big_a, big_b):
                        if i < len(lst):
                            lst[i]()
                            cnt += 1
                            if cnt % every == 0 and ei < len(extra):
                                extra[ei]()
                                ei += 1
                while ei < len(extra):
                    extra[ei]()
                    ei += 1

            # ---------------- the software pipeline ----------------
            # iteration i emits: phase A = o(i-2) x qk(i) with pu(i-1)
            # sprinkled; phase B = v(i) x logits(i).
            e_hist = {}
            vplus_hist = {}
            attn_hist = {}

            for it in range(PAIRS + 2):
                if it < PAIRS:
                    q_t, k_t, v_t = prefetched if it == 0 else load_inputs(it)
                    qsT = qkt.tile([128, 2, XT, C], fp8, tag="qsT")
                    khT = qkt.tile([128, 2, XT, C], fp8, tag="khT")
                    vplus = vplus_bufs[it % 2]
                    vplus_hist[it] = vplus
                    e_t = e_bufs[it % 2]
                    e_hist[it] = e_t
                    plan0 = [(psQK, "qk"), (psQK, "qk"), (psOV, "ov"),
                             (psOV, "ov"), (psL, "pl"), (psL, "pl"),
                             (psU, "pu"), (psU, "pu")]
                    qk_th = qk_thunks(q_t, k_t, qsT, khT,
                                      kp_first=(it == 1),
                                      pool_plan=(plan0 if it == 0 else None))
                    v_th = v_thunks(v_t, vplus)
                    lg_th = logits_thunks(qsT, khT, e_t)
                else:
                    qk_th, v_th, lg_th = [], [], []

                if it >= 2:
                    attn = attn_hist.pop(it - 2)
                    o_th = o_thunks(attn, (it - 2) * 2)
                else:
                    o_th = []

                if 1 <= it <= PAIRS:
                    e_p = e_hist.pop(it - 1)
                    vplus_p = vplus_hist.pop(it - 1)
                    attn_n = mid.tile([128, NB, 2, X], bf16, tag="attn")
                    attn_hist[it - 1] = attn_n
                    pu_th = pu_thunks(e_p, vplus_p, attn_n)
                else:
                    pu_th = []

                if it == 0:
                    # warmup: the PE queue is FIFO, so nothing that waits on
                    # late DMA may be emitted early. All eight k-tile-01
                    # matmuls need only the first two input chunks -> run
                    # them first (their PSUM tiles are spread over all four
                    # pools, see pool_plan), then the k-tile-23 matmuls,
                    # then pair the v matmuls against the logits work.
                    for t in qk_th[0::2]:
                        t()
                    for t in qk_th[1::2]:
                        t()
                    interleave(v_th, lg_th)
                elif it == 1:
                    # pair-1 inputs are still streaming in; bridge the wait
                    # with pu(0), whose inputs are already on-chip.
                    for t in pu_th:
                        t()
                    dr_all = qk_th[:8] + lg_th[:6] + qk_th[8:] + lg_th[6:]
                    interleave(v_th, dr_all)
                else:
                    interleave(o_th, qk_th, pu_th, every=4)
                    interleave(v_th, lg_th)

    nc.compile()
    return nc


def _get_program():
    if "nc" not in _CACHE:
        _CACHE["nc"] = _build_program()
    return _CACHE["nc"]


def _make_in_maps(query, key_, value, Wq, Wk, Wv, Wo, q_emb, k_emb, v_emb):
    import ml_dtypes
    bf16 = ml_dtypes.bfloat16
    fp8 = ml_dtypes.float8_e4m3

    def q8(a, scale):
        return np.ascontiguousarray(
            np.clip(a * np.float32(scale), -240, 240).astype(fp8))

    # Head-pair channel permutation: 128-blocks hold heads [0,1],[3,2],
    # [4,5],[7,6]; the first head of a block owns partitions/block-columns
    # 0:64, the second 64:128. Applied to Wq/Wk/Wv output channels and Wo
    # input channels so the merged head-pair matmuls read/write contiguous
    # 128-blocks.
    vperm = np.arange(C).reshape(C // 128, 2, QK)[:, [0, 1], :].copy()
    vperm[1::2] = vperm[1::2][:, [1, 0], :]
    rowperm = vperm.reshape(-1)

    # Scale plan (logits accumulate 4096x, undone in the exp activation):
    #   qin/kin fp8 at 1x; Wq/Wk fp8 at 64x -> psum 64x; evac scale 0.25
    #   -> qsT/khT fp8 at 16x; head-pair term 256x = 4096 * (1/16 softmax).
    #   q_emb fp8 at 16x (pairs with qsT); k_emb fp8 at 256x (pairs with khT,
    #   no softmax scale on the k.ke term).
    wqt = q8(Wq[rowperm].T, 64.0)
    wkt = q8(Wk[rowperm].T, 64.0)
    wvt = np.ascontiguousarray(Wv[rowperm].T.astype(bf16))
    wot = np.ascontiguousarray(Wo.T[rowperm].astype(bf16))
    qe2 = q8(np.concatenate([q_emb, q_emb], axis=1), 16.0)
    ke2 = q8(np.concatenate([k_emb, k_emb], axis=1), 256.0)
    # ve dup'd over the two head-halves and the two w columns: [128, 2, X]
    ve1 = np.concatenate([v_emb.T, v_emb.T], axis=0)          # (128, X)
    vet = np.ascontiguousarray(
        np.broadcast_to(ve1[:, None, :], (128, 2, X)).astype(np.float32))

    def shard8(a, ws):
        # (C, X, WC) -> [pair, c, w, x] contiguous fp8
        return q8(
            a[:, :, ws].reshape(C, X, PAIRS, 2).transpose(2, 0, 3, 1), 1.0)

    def shardb(a, ws):
        return np.ascontiguousarray(
            a[:, :, ws].reshape(C, X, PAIRS, 2).transpose(2, 0, 3, 1).astype(bf16))

    in_maps = []
    for c in range(N_CORES):
        ws = slice(c * WC, (c + 1) * WC)
        in_maps.append({
            "qin": shard8(query, ws),
            "kin": shard8(key_, ws),
            "vin": shardb(value, ws),
            "wqt": wqt, "wkt": wkt, "wvt": wvt, "wot": wot,
            "qe2": qe2, "ke2": ke2, "vet": vet,
        })
    return in_maps


def _run(in_maps, trace=False):
    from concourse.bass_utils import run_bass_kernel_spmd
    nc = _get_program()
    return run_bass_kernel_spmd(nc, in_maps, list(range(N_CORES)), trace=trace)


def kernel(query, key_, value, Wq, Wk, Wv, Wo, q_emb, k_emb, v_emb):
    args = (query, key_, value, Wq, Wk, Wv, Wo, q_emb, k_emb, v_emb)
    in_maps = _make_in_maps(*[np.ascontiguousarray(a, np.float32) for a in args])
    res = _run(in_maps, trace=False)
    out = np.empty((C, X, W), np.float32)
    for c in range(N_CORES):
        out[:, :, c * WC:(c + 1) * WC] = \
            res.results[c]["out"].astype(np.float32).transpose(0, 2, 1)
    return out


# revision 40
# speedup vs baseline: 1.1864x; 1.0031x over previous
# Trainium2 Bass kernel for nn_AxialAttention (8 NeuronCores, W-parallel).
#
# Sharding: the W axis (axis=2, the vmapped axis) is split into 8 contiguous
# slices of 32 columns, one per core. Every part of the computation (the four
# 1x1-conv GEMMs, the per-(head, w) axial attention, the embedding terms) is
# independent across w, so there are no collectives; the small weight matrices
# and embedding tables are replicated to every core.
#
# Per-core math for one w column (all heads):
#   qsT[x, (h c)] = query[:, :, w].T @ Wq.T     (fp8 DoubleRow, K=256/pass)
#   khT[x, (h c)] = key_[:, :, w].T @ Wk.T      (fp8 DoubleRow)
#   vh [(h c), x] = Wv @ value[:, :, w]         (bf16)
#   logits_h[C, c] = khT_h.T @ qsT_h + qe.T @ qsT + ke.T @ khT
#     (scales: qin/kin fp8 at 1x, Wq/Wk fp8 at 64x, qsT/khT stored fp8 at
#      16x -> logits accumulate 4096*(true logits); softmax's 1/sqrt(256)
#      is folded into that 4096)
#   E = exp(logits / 4096)        (max-subtraction unnecessary: |logits|<~2)
#   U_h = E_h.T @ [vh_h + ve | 1]          (ones column gives the softmax
#   attn_h = U_h[:, :256] / U_h[:, 256]     denominator for free)
#   out[:, :, w] = Wo @ attn                (bf16 GEMM, output stored bf16)
#
# fp8 (e4m3) is used only where quantization noise lands pre-softmax (the
# q/k path); the v/o path must stay bf16 (fp8 there measures 3.5e-2 vs the
# 2e-2 gate).
#
# Scheduling: the PE weight-load port is a co-bottleneck with the matmul
# stream, so the kernel is organized around hiding LDWEIGHTS:
#  - Heads are packed two-per-128-block (block order [0,1],[3,2],[4,5],[7,6]
#    via a host-side channel permutation of Wq/Wk/Wv/Wo), so each head-pair's
#    logits term is ONE full-array DoubleRow matmul (cross-head products land
#    in the partition half the other head doesn't use) and each head-pair's
#    attention-x-values product is ONE full-array matmul (the unused halves
#    of the exp tile are kept hard-zero by masked exp writes, so cross-head
#    rows contribute exactly 0). This halves both matmul and LDWEIGHTS count
#    versus 64-wide per-head tiles.
#  - fp8 DoubleRow matmuls (256-column LDWEIGHTS that can't hide behind
#    another DR matmul) are emitted strictly interleaved with bf16 matmuls
#    (o/v projections) whose streams cover the load: phase A of iteration i
#    alternates o(i-2) with the q/k projections of i plus the attention
#    matmuls of i-1; phase B alternates the v projection of i with the
#    logits matmuls of i.
#  - Output is written bf16 on the otherwise-idle GpSimd DGE ring; PSUM->SBUF
#    evacuations are split across the Scalar/Vector engines.

import numpy as np

H = 8          # heads
QK = 64        # per-head qk/vo channels
C = 512        # io channels
X = 256        # spatial H (attention contraction axis)
W = 256        # spatial W (vmapped axis, sharded)
N_CORES = 8
WC = W // N_CORES   # w columns per core
PAIRS = WC // 2
NB = 4         # head-pair blocks (2 heads per 128 channels)

_CACHE = {}


def _build_program():
    import concourse.mybir as mybir
    import concourse.tile as tile
    from concourse import bacc

    f32 = mybir.dt.float32
    bf16 = mybir.dt.bfloat16
    fp8 = mybir.dt.float8e4
    AF = mybir.ActivationFunctionType
    DR = mybir.MatmulPerfMode.DoubleRow

    nc = bacc.Bacc("TRN2", target_bir_lowering=False, debug=False,
                   num_devices=N_CORES)

    qin = nc.dram_tensor("qin", [PAIRS, C, 2, X], fp8, kind="ExternalInput").ap()
    kin = nc.dram_tensor("kin", [PAIRS, C, 2, X], fp8, kind="ExternalInput").ap()
    vin = nc.dram_tensor("vin", [PAIRS, C, 2, X], bf16, kind="ExternalInput").ap()
    wqt = nc.dram_tensor("wqt", [C, C], fp8, kind="ExternalInput").ap()
    wkt = nc.dram_tensor("wkt", [C, C], fp8, kind="ExternalInput").ap()
    wvt = nc.dram_tensor("wvt", [C, C], bf16, kind="ExternalInput").ap()
    wot = nc.dram_tensor("wot", [C, C], bf16, kind="ExternalInput").ap()
    qe2 = nc.dram_tensor("qe2", [X, 2 * QK], fp8, kind="ExternalInput").ap()
    ke2 = nc.dram_tensor("ke2", [X, 2 * QK], fp8, kind="ExternalInput").ap()
    vet = nc.dram_tensor("vet", [128, 2, X], f32, kind="ExternalInput").ap()
    out = nc.dram_tensor("out", [C, WC, X], bf16, kind="ExternalOutput").ap()

    KT = C // 128   # 4 contraction tiles of the channel dim
    XT = X // 128   # 2 tiles of the spatial-x dim

    with tile.TileContext(nc) as tc:
        with (
            tc.tile_pool(name="consts", bufs=1) as consts,
            tc.tile_pool(name="inp", bufs=4) as inp,
            tc.tile_pool(name="qkt", bufs=2) as qkt,
            tc.tile_pool(name="mid", bufs=2) as mid,
            tc.tile_pool(name="small", bufs=8) as small,
            tc.tile_pool(name="psQK", bufs=2, space="PSUM") as psQK,
            tc.tile_pool(name="psOV", bufs=2, space="PSUM") as psOV,
            tc.tile_pool(name="psL", bufs=2, space="PSUM") as psL,
            tc.tile_pool(name="psU", bufs=2, space="PSUM") as psU,
        ):
            def load_inputs(pair):
                q_t = inp.tile([128, KT, 2, X], fp8, tag="q_t")
                nc.sync.dma_start(
                    q_t[:], qin[pair].rearrange("(kt p) w x -> p kt (w x)", p=128))
                k_t = inp.tile([128, KT, 2, X], fp8, tag="k_t")
                nc.sync.dma_start(
                    k_t[:], kin[pair].rearrange("(kt p) w x -> p kt (w x)", p=128))
                v_t = inp.tile([128, KT, 2, X], bf16, tag="v_t")
                nc.sync.dma_start(
                    v_t[:], vin[pair].rearrange("(kt p) w x -> p kt (w x)", p=128))
                return q_t, k_t, v_t

            # pair-0 inputs first so the PE can start ASAP; each dma_start
            # costs ~600ns of issue time on its DGE queue, so loads are
            # halved (not quartered) and ordered by first use: q/k k-tiles
            # 0-1 feed the first DR matmuls, v trails.
            q0 = inp.tile([128, KT, 2, X], fp8, tag="q_t")
            k0 = inp.tile([128, KT, 2, X], fp8, tag="k_t")
            v0 = inp.tile([128, KT, 2, X], bf16, tag="v_t")
            qr0 = qin[0].rearrange("(h p) w x -> p h (w x)", p=128)
            kr0 = kin[0].rearrange("(h p) w x -> p h (w x)", p=128)
            vr0 = vin[0].rearrange("(h p) w x -> p h (w x)", p=128)
            nc.sync.dma_start(q0[:, 0:2, :, :], qr0[:, 0:2, :])
            nc.sync.dma_start(k0[:, 0:2, :, :], kr0[:, 0:2, :])
            nc.sync.dma_start(q0[:, 2:4, :, :], qr0[:, 2:4, :])
            nc.sync.dma_start(k0[:, 2:4, :, :], kr0[:, 2:4, :])
            nc.sync.dma_start(v0[:, 0:2, :, :], vr0[:, 0:2, :])
            nc.sync.dma_start(v0[:, 2:4, :, :], vr0[:, 2:4, :])
            prefetched = (q0, k0, v0)

            # wq first (the first matmul needs only its kt 0-1 half), then wk.
            wq_sb = consts.tile([128, KT, C], fp8)
            wqr = wqt.rearrange("(kt p) o -> p kt o", p=128)
            nc.scalar.dma_start(wq_sb[:, 0:2, :], wqr[:, 0:2, :])
            nc.scalar.dma_start(wq_sb[:, 2:4, :], wqr[:, 2:4, :])
            wk_sb = consts.tile([128, KT, C], fp8)
            nc.scalar.dma_start(wk_sb[:], wkt.rearrange("(kt p) o -> p kt o", p=128))
            # later-used constants go on the idle GpSimd ring so the scalar
            # ring only carries what the first matmuls need; ordered by first
            # use (emb tables feed the warmup logits, wv is split so the
            # first v matmul only waits half of it, wo isn't needed until
            # iteration 2).
            ke_sb = consts.tile([128, XT, 2 * QK], fp8)
            nc.gpsimd.dma_start(ke_sb[:], ke2.rearrange("(xt p) m -> p xt m", p=128))
            qe_sb = consts.tile([128, XT, 2 * QK], fp8)
            nc.gpsimd.dma_start(qe_sb[:], qe2.rearrange("(xt p) m -> p xt m", p=128))
            wv_sb = consts.tile([128, KT, C], bf16)
            wvr = wvt.rearrange("(kt p) o -> p kt o", p=128)
            nc.gpsimd.dma_start(wv_sb[:, 0:2, :], wvr[:, 0:2, :])
            nc.gpsimd.dma_start(wv_sb[:, 2:4, :], wvr[:, 2:4, :])
            ve_sb = consts.tile([128, 2, X], f32)   # dup'd over head-half and w
            nc.gpsimd.dma_start(ve_sb[:], vet[:])
            wo_sb = consts.tile([128, KT, C], bf16)
            nc.gpsimd.dma_start(wo_sb[:], wot.rearrange("(kt p) o -> p kt o", p=128))

            # vplus double-buffer with the ones columns filled exactly once
            # (they never change; pool rotation would clobber them).
            vplus_bufs = []
            for b in range(2):
                vb = mid.tile([128, NB, 2, X + 2], bf16, tag=f"vplus{b}")
                nc.vector.memset(vb[:, :, :, X:X + 2], 1.0)
                vplus_bufs.append(vb)

            # exp tiles: the off-half of each head-pair block must stay ZERO
            # so the merged (full-K) attention matmul gets exactly-zero
            # cross-head contributions. Memset once; exp only writes the
            # valid half of each block.
            e_bufs = []
            for b in range(2):
                eb = mid.tile([128, 2, NB, 128], bf16, tag=f"e{b}")
                nc.vector.memset(eb[0:QK, :, :, QK:128], 0.0)
                nc.vector.memset(eb[QK:128, :, :, 0:QK], 0.0)
                e_bufs.append(eb)

            # ---------------- per-iteration emission helpers ----------------
            # Each helper returns a list of thunks; calling a thunk emits ONE
            # PE matmul (plus any trailing non-PE ops tied to it). Emission
            # order = scheduler priority = (modulo readiness) PE issue order.

            def qk_thunks(q_t, k_t, qsT, khT, kp_first=False, pool_plan=None):
                # kp_first: emit [q_kp0, k_kp0, q_kp1, k_kp1] per (wi, xt) so
                # the k-tile-23 matmuls trail the k-tile-01 ones, matching
                # the DMA arrival order of the input chunks (warmup only).
                # pool_plan: per-group (pool, tag) override so iteration 0
                # can spread its 8 groups across all four PSUM pools.
                th = []
                gi = 0
                for wi in range(2):
                    for xt in range(XT):
                        for which, src, wsb, dstT in (("q", q_t, wq_sb, qsT),
                                                      ("k", k_t, wk_sb, khT)):
                            pool, tag = (pool_plan[gi] if pool_plan
                                         else (psQK, "qk"))
                            gi += 1
                            cell = {}
                            def t0(cell=cell, src=src, wsb=wsb, wi=wi, xt=xt,
                                   pool=pool, tag=tag):
                                p = pool.tile([128, C], f32, tag=tag)
                                cell["p"] = p
                                nc.tensor.matmul(
                                    p[:],
                                    src[:, 0:2, wi, xt * 128:(xt + 1) * 128],
                                    wsb[:, 0:2, :],
                                    start=True, stop=False, perf_mode=DR)
                            def t1(cell=cell, src=src, wsb=wsb, dstT=dstT,
                                   which=which, wi=wi, xt=xt):
                                p = cell["p"]
                                nc.tensor.matmul(
                                    p[:],
                                    src[:, 2:4, wi, xt * 128:(xt + 1) * 128],
                                    wsb[:, 2:4, :],
                                    start=False, stop=True, perf_mode=DR)
                                # high priority: the evac is the PSUM-bank
                                # release; with only 2 qk banks a queue delay
                                # here stalls the PE two groups later.
                                with tc.high_priority():
                                    if which == "q":
                                        nc.scalar.activation(
                                            dstT[:, wi, xt, :], p[:], AF.Copy,
                                            scale=0.25)
                                    else:
                                        nc.vector.tensor_scalar_mul(
                                            dstT[:, wi, xt, :], p[:], 0.25)
                            th += [t0, t1]
                if kp_first:
                    # [qa qb ka kb] -> [qa ka qb kb] within each (wi, xt)
                    th = [th[g + j] for g in range(0, len(th), 4)
                          for j in (0, 2, 1, 3)]
                return th

            def proj_thunks(src, wsb, on_group_done):
                # generic 4x4 bf16 projection: out-block ot accumulates kt 0..3
                th = []
                for ot in range(KT):
                    cell = {}
                    for kt in range(KT):
                        def f(cell=cell, ot=ot, kt=kt):
                            if kt == 0:
                                cell["p"] = psOV.tile([128, 2, X], f32, tag="ov", name="pov")
                            nc.tensor.matmul(
                                cell["p"][:],
                                wsb[:, kt, ot * 128:(ot + 1) * 128],
                                src[:, kt, :, :],
                                start=(kt == 0), stop=(kt == KT - 1))
                            if kt == KT - 1:
                                on_group_done(ot, cell["p"])
                        th.append(f)
                return th

            def v_thunks(v_t, vplus):
                def done(ot, psum):
                    nc.vector.tensor_add(
                        vplus[:, ot, :, 0:X], psum[:], ve_sb[:])
                return proj_thunks(v_t, wv_sb, done)

            def o_thunks(attn, w0):
                def done(ot, psum):
                    ob = small.tile([128, 2, X], bf16, tag="ob")
                    if ot % 2 == 0:
                        nc.scalar.activation(ob[:, 0, :], psum[:, 0, :], AF.Copy)
                        nc.vector.tensor_copy(ob[:, 1, :], psum[:, 1, :])
                    else:
                        nc.scalar.activation(ob[:, 1, :], psum[:, 1, :], AF.Copy)
                        nc.vector.tensor_copy(ob[:, 0, :], psum[:, 0, :])
                    # one HWDGE queue caps at ~40-44 GB/s, which barely
                    # covers the 512KB/iteration of output: send 1 of 4
                    # blocks to the (post-startup idle) scalar ring so the
                    # gpsimd ring stays at ~2/3 of its cap, without loading
                    # the scalar engine queue with more issue instructions.
                    ring = nc.scalar if ot == 1 else nc.gpsimd
                    ring.dma_start(
                        out[ot * 128:(ot + 1) * 128, w0:w0 + 2, :], ob[:])
                return proj_thunks(attn, wo_sb, done)

            def logits_thunks(qsT, khT, e_t):
                # per wi: ke, qe emb DR matmuls + NB merged head-pair DR
                # matmuls accumulating into pl, then masked exp -> e_t.
                th = []
                for wi in range(2):
                    cell = {}
                    def t_ke(cell=cell, wi=wi):
                        pl = psL.tile([128, NB, 128], f32, tag="pl", name="pl")
                        cell["pl"] = pl
                        nc.tensor.matmul(
                            pl[:], ke_sb[:], khT[:, wi, :, :],
                            start=True, stop=False, perf_mode=DR)
                    def t_qe(cell=cell, wi=wi):
                        nc.tensor.matmul(
                            cell["pl"][:], qe_sb[:], qsT[:, wi, :, :],
                            start=False, stop=False, perf_mode=DR)
                    th += [t_ke, t_qe]
                    for p in range(NB):
                        def t_hp(cell=cell, wi=wi, p=p):
                            pl = cell["pl"]
                            nc.tensor.matmul(
                                pl[:, p, :],
                                khT[:, wi, :, p * 128:(p + 1) * 128],
                                qsT[:, wi, :, p * 128:(p + 1) * 128],
                                start=False, stop=(p == NB - 1),
                                perf_mode=DR)
                            if p == NB - 1:
                                nc.scalar.activation(
                                    e_t[0:QK, wi, :, 0:QK],
                                    pl[0:QK, :, 0:QK], AF.Exp,
                                    scale=1.0 / 4096.0)
                                nc.scalar.activation(
                                    e_t[QK:128, wi, :, QK:128],
                                    pl[QK:128, :, QK:128], AF.Exp,
                                    scale=1.0 / 4096.0)
                        th.append(t_hp)
                return th

            def pu_thunks(e_t, vplus, attn):
                # p-major: the o projection consumes attn k-tile 0 (= block
                # p=0, both wi) first, so evacuate in that order.
                th = []
                for p in range(NB):
                    for wi in range(2):
                        def t(wi=wi, p=p):
                            pu = psU.tile([128, X + 2], f32, tag="pu")
                            nc.tensor.matmul(
                                pu[:],
                                e_t[:, wi, p, :],
                                vplus[:, p, wi, :],
                                start=True, stop=True)
                            # high priority: recip -> scale is the psU bank
                            # release chain; a queue delay stalls the pu
                            # matmul two tiles later.
                            with tc.high_priority():
                                recip = small.tile([128, 1], f32, tag="recip")
                                nc.vector.reciprocal(recip[:], pu[:, X:X + 1])
                                if (2 * wi + p) % 4 != 3:  # scalar is lighter
                                    nc.scalar.activation(
                                        attn[:, p, wi, :],
                                        pu[:, 0:X], AF.Copy, scale=recip[:])
                                else:
                                    nc.vector.tensor_scalar_mul(
                                        attn[:, p, wi, :], pu[:, 0:X], recip[:])
                        th.append(t)
                return th

            def interleave(big_a, big_b, extra=(), every=4):
                # alternate big_a/big_b; insert one `extra` thunk after every
                # `every` big thunks.
                n = max(len(big_a), len(big_b))
                ei = 0
                cnt = 0
                for i in range(n):
                    for lst in (big_a, big_b):
                        if i < len(lst):
                            lst[i]()
                            cnt += 1
                            if cnt % every == 0 and ei < len(extra):
                                extra[ei]()
                                ei += 1
                while ei < len(extra):
                    extra[ei]()
                    ei += 1

            # ---------------- the software pipeline ----------------
            # iteration i emits: phase A = o(i-2) x qk(i) with pu(i-1)
            # sprinkled; phase B = v(i) x logits(i).
            e_hist = {}
            vplus_hist = {}
            attn_hist = {}

            for it in range(PAIRS + 2):
                if it < PAIRS:
                    q_t, k_t, v_t = prefetched if it == 0 else load_inputs(it)
                    qsT = qkt.tile([128, 2, XT, C], fp8, tag="qsT")
                    khT = qkt.tile([128, 2, XT, C], fp8, tag="khT")
                    vplus = vplus_bufs[it % 2]
                    vplus_hist[it] = vplus
                    e_t = e_bufs[it % 2]
                    e_hist[it] = e_t
                    plan0 = [(psQK, "qk"), (psQK, "qk"), (psOV, "ov"),
                             (psOV, "ov"), (psL, "pl"), (psL, "pl"),
                             (psU, "pu"), (psU, "pu")]
                    qk_th = qk_thunks(q_t, k_t, qsT, khT,
                                      kp_first=(it == 1),
                                      pool_plan=(plan0 if it == 0 else None))
                    v_th = v_thunks(v_t, vplus)
                    lg_th = logits_thunks(qsT, khT, e_t)
                else:
                    qk_th, v_th, lg_th = [], [], []

                if it >= 2:
                    attn = attn_hist.pop(it - 2)
                    o_th = o_thunks(attn, (it - 2) * 2)
                else:
                    o_th = []

                if 1 <= it <= PAIRS:
                    e_p = e_hist.pop(it - 1)
                    vplus_p = vplus_hist.pop(it - 1)
                    attn_n = mid.tile([128, NB, 2, X], bf16, tag="attn")
                    attn_hist[it - 1] = attn_n
                    pu_th = pu_thunks(e_p, vplus_p, attn_n)
                else:
                    pu_th = []

                if it == 0:
                    # warmup: the PE queue is FIFO, so nothing that waits on
                    # late DMA may be emitted early. All eight k-tile-01
                    # matmuls need only the first two input chunks -> run
                    # them first (their PSUM tiles are spread over all four
                    # pools, see pool_plan), then the k-tile-23 matmuls,
                    # then pair the v matmuls against the logits work.
                    for t in qk_th[0::2]:
                        t()
                    for t in qk_th[1::2]:
                        t()
                    interleave(v_th, lg_th)
                elif it == 1:
                    # pair-1 inputs are still streaming in; bridge the wait
                    # with pu(0), whose inputs are already on-chip.
                    for t in pu_th:
                        t()
                    dr_all = qk_th[:8] + lg_th[:6] + qk_th[8:] + lg_th[6:]
                    interleave(v_th, dr_all)
                else:
                    interleave(o_th, qk_th, pu_th, every=4)
                    interleave(v_th, lg_th)

    nc.compile()
    return nc


def _get_program():
    if "nc" not in _CACHE:
        _CACHE["nc"] = _build_program()
    return _CACHE["nc"]


def _make_in_maps(query, key_, value, Wq, Wk, Wv, Wo, q_emb, k_emb, v_emb):
    import ml_dtypes
    bf16 = ml_dtypes.bfloat16
    fp8 = ml_dtypes.float8_e4m3

    def q8(a, scale):
        return np.ascontiguousarray(
            np.clip(a * np.float32(scale), -240, 240).astype(fp8))

    # Head-pair channel permutation: 128-blocks hold heads [0,1],[3,2],
    # [4,5],[7,6]; the first head of a block owns partitions/block-columns
    # 0:64, the second 64:128. Applied to Wq/Wk/Wv output channels and Wo
    # input channels so the merged head-pair matmuls read/write contiguous
    # 128-blocks.
    vperm = np.arange(C).reshape(C // 128, 2, QK)[:, [0, 1], :].copy()
    vperm[1::2] = vperm[1::2][:, [1, 0], :]
    rowperm = vperm.reshape(-1)

    # Scale plan (logits accumulate 4096x, undone in the exp activation):
    #   qin/kin fp8 at 1x; Wq/Wk fp8 at 64x -> psum 64x; evac scale 0.25
    #   -> qsT/khT fp8 at 16x; head-pair term 256x = 4096 * (1/16 softmax).
    #   q_emb fp8 at 16x (pairs with qsT); k_emb fp8 at 256x (pairs with khT,
    #   no softmax scale on the k.ke term).
    wqt = q8(Wq[rowperm].T, 64.0)
    wkt = q8(Wk[rowperm].T, 64.0)
    wvt = np.ascontiguousarray(Wv[rowperm].T.astype(bf16))
    wot = np.ascontiguousarray(Wo.T[rowperm].astype(bf16))
    qe2 = q8(np.concatenate([q_emb, q_emb], axis=1), 16.0)
    ke2 = q8(np.concatenate([k_emb, k_emb], axis=1), 256.0)
    # ve dup'd over the two head-halves and the two w columns: [128, 2, X]
    ve1 = np.concatenate([v_emb.T, v_emb.T], axis=0)          # (128, X)
    vet = np.ascontiguousarray(
        np.broadcast_to(ve1[:, None, :], (128, 2, X)).astype(np.float32))

    def shard8(a, ws):
        # (C, X, WC) -> [pair, c, w, x] contiguous fp8
        return q8(
            a[:, :, ws].reshape(C, X, PAIRS, 2).transpose(2, 0, 3, 1), 1.0)

    def shardb(a, ws):
        return np.ascontiguousarray(
            a[:, :, ws].reshape(C, X, PAIRS, 2).transpose(2, 0, 3, 1).astype(bf16))

    in_maps = []
    for c in range(N_CORES):
        ws = slice(c * WC, (c + 1) * WC)
        in_maps.append({
            "qin": shard8(query, ws),
            "kin": shard8(key_, ws),
            "vin": shardb(value, ws),
            "wqt": wqt, "wkt": wkt, "wvt": wvt, "wot": wot,
            "qe2": qe2, "ke2": ke2, "vet": vet,
        })
    return in_maps


def _run(in_maps, trace=False):
    from concourse.bass_utils import run_bass_kernel_spmd
    nc = _get_program()
    return run_bass_kernel_spmd(nc, in_maps, list(range(N_CORES)), trace=trace)


def kernel(query, key_, value, Wq, Wk, Wv, Wo, q_emb, k_emb, v_emb):
    args = (query, key_, value, Wq, Wk, Wv, Wo, q_emb, k_emb, v_emb)
    in_maps = _make_in_maps(*[np.ascontiguousarray(a, np.float32) for a in args])
    res = _run(in_maps, trace=False)
    out = np.empty((C, X, W), np.float32)
    for c in range(N_CORES):
        out[:, :, c * WC:(c + 1) * WC] = \
            res.results[c]["out"].astype(np.float32).transpose(0, 2, 1)
    return out
